# revision 1
# baseline (speedup 1.0000x reference)
"""Batched normalized-gram kernel for 8 TRN2 NeuronCores.

reference:  x (64, 2, 512, 512) fp32
    x0 = x[:, 0]                               (B=64, V=512, F=512)
    n  = sqrt(sum(x0^2, axis=(0, 2)))          (V,)
    out[b] = (x0[b] @ x0[b].T) / outer(n, n)   (B, V, V)

Since gram[b,i,j]/(n_i n_j) == (x0[b,i,:]/n_i) . (x0[b,j,:]/n_j), the host
prescales rows by 1/n once and the device work is a pure batched symmetric
matmul out[b] = y[b] @ y[b].T.

Device-side tricks:
  * operands shipped as fp16 (|y| <= ~0.05, comfortably normal) — halves
    input DMA, full-rate PE, fp32 PSUM accumulation keeps rel err ~2e-4.
  * out[b] is symmetric, and the reference is *exactly* symmetric (same
    products, same summation order), so the device computes only the upper
    block-triangle (row-block mi covers columns mi*128..511) and the host
    mirrors the lower blocks.  -37.5% output DMA, -37.5% PE work.

Sharding: data-parallel over batch — 8 batches per core, no collectives.
"""

import numpy as np

B, T, V, F = 64, 2, 512, 512
NCORES = 8
BPC = B // NCORES  # batches per core
NBLK = V // 128  # 4 row-blocks

_NC = None


def _build_nc():
    import concourse.mybir as mybir
    import concourse.tile as tile
    from concourse import bacc

    f32 = mybir.dt.float32
    f16 = mybir.dt.float16

    nc = bacc.Bacc(target_bir_lowering=False)
    yT = nc.declare_dram_parameter("yT", [BPC, F, V], f16, isOutput=False)
    out = nc.declare_dram_parameter("out", [BPC, V, V], f32, isOutput=True)

    # upper-triangle segment offsets inside the per-batch output tile:
    # row-block mi holds columns mi*128..511 (N = 512 - 128*mi)
    seg_off = [0]
    for mi in range(NBLK):
        seg_off.append(seg_off[-1] + V - 128 * mi)
    seg_total = seg_off[-1]  # 1280

    with tile.TileContext(nc) as tc:
        with (
            tc.tile_pool(name="inp", bufs=10) as inp_pool,
            tc.tile_pool(name="psum", bufs=8, space="PSUM") as psum_pool,
            tc.tile_pool(name="outp", bufs=3) as out_pool,
        ):
            for b in range(BPC):
                # yT[b] is (F, V): four partition-chunks of [128, V], one
                # tile each so matmuls depend only on the chunk they read.
                # Input DMAs ride the SP HWDGE ring; output DMAs ride the
                # ACT ring — two independent FIFOs running concurrently.
                chunks = []
                for ki in range(NBLK):
                    ck = inp_pool.tile([128, V], f16, tag="ck")
                    nc.sync.dma_start(
                        out=ck, in_=yT[b, ki * 128 : (ki + 1) * 128, :]
                    )
                    chunks.append(ck)
                ot = out_pool.tile([128, seg_total], f32)
                for mi in range(NBLK):
                    n_cols = V - 128 * mi
                    ps = psum_pool.tile([128, n_cols], f32, tag="ps")
                    for ki in range(NBLK):
                        nc.tensor.matmul(
                            ps,
                            lhsT=chunks[ki][:, mi * 128 : (mi + 1) * 128],
                            rhs=chunks[ki][:, mi * 128 :],
                            start=(ki == 0),
                            stop=(ki == NBLK - 1),
                        )
                    seg = ot[:, seg_off[mi] : seg_off[mi] + n_cols]
                    nc.vector.tensor_copy(out=seg, in_=ps)
                    nc.scalar.dma_start(
                        out=out[b, mi * 128 : (mi + 1) * 128, mi * 128 :],
                        in_=seg,
                    )
    if not nc.is_finalized():
        nc.finalize()
    return nc


def _get_nc():
    global _NC
    if _NC is None:
        _NC = _build_nc()
    return _NC


def _prep_shards(x: np.ndarray) -> np.ndarray:
    x = np.ascontiguousarray(np.asarray(x, dtype=np.float32))
    x0 = x[:, 0]  # (B, V, F)
    ss = np.einsum("bvf,bvf->v", x0, x0, optimize=True)
    inv_n = (1.0 / np.sqrt(ss)).astype(np.float32)
    y = x0 * inv_n[None, :, None]
    # (B, F, V) contiguous so each core's operand streams with unit stride
    return np.ascontiguousarray(np.transpose(y, (0, 2, 1)).astype(np.float16))


def kernel(x: np.ndarray, _trace: bool = False, _trace_out: list | None = None):
    from concourse.bass_utils import run_bass_kernel_spmd

    yT = _prep_shards(x)
    nc = _get_nc()
    in_maps = [{"yT": yT[c * BPC : (c + 1) * BPC]} for c in range(NCORES)]
    res = run_bass_kernel_spmd(
        nc, in_maps, core_ids=list(range(NCORES)), trace=_trace
    )
    if _trace_out is not None:
        _trace_out.append(res)
    full = np.concatenate(
        [np.asarray(res.results[c]["out"]) for c in range(NCORES)], axis=0
    )
    # device wrote only the upper block-triangle; mirror it down
    for mi in range(NBLK):
        for nj in range(mi + 1, NBLK):
            full[:, nj * 128 : (nj + 1) * 128, mi * 128 : (mi + 1) * 128] = (
                np.swapaxes(
                    full[:, mi * 128 : (mi + 1) * 128, nj * 128 : (nj + 1) * 128],
                    1,
                    2,
                )
            )
    return full



# revision 2
# speedup vs baseline: 1.2074x; 1.2074x over previous
"""Batched normalized-gram kernel for 8 TRN2 NeuronCores.

reference:  x (64, 2, 512, 512) fp32
    x0 = x[:, 0]                               (B=64, V=512, F=512)
    n  = sqrt(sum(x0^2, axis=(0, 2)))          (V,)
    out[b] = (x0[b] @ x0[b].T) / outer(n, n)   (B, V, V)

gram[b,i,j]/(n_i n_j) == (x0[b,i,:]/n_i) . (x0[b,j,:]/n_j), so the host
prescales rows by 1/n once and the device work is a pure batched symmetric
matmul out[b] = y[b] @ y[b].T.

Device-side structure (per core, 8 batches):
  * operands shipped as fp16 — halves input DMA, full-rate PE, fp32 PSUM.
  * output is symmetric: device computes only the upper block-triangle
    (row-block mi covers columns mi*128..511), host mirrors the rest.
  * ONE input DMA per batch: the host pre-interleaves y[b].T into a
    [128, 2048] layout (z[b, p, ki*512+v] = yT[b, ki*128+p, v]) so the
    whole batch streams as 128 x 4KiB contiguous descriptors.  Keeps the
    SP sequencer (~600ns config per DMA) far ahead of the PE.
  * ONE output DMA per batch: the 1280-column packed triangle is staged
    in SBUF as fp16 (halves output DMA) and unpacked/mirrored on host.
  * PSUM->SBUF copies alternate DVE / ACT so neither engine bottlenecks.
  * dummy warm-up matmuls on a zeroed tile run while batch 0's input DMA
    is in flight: PE starts busy at t~0, HAM un-throttles to 2.4 GHz by
    the time real matmuls begin.

Sharding: data-parallel over batch — 8 batches per core, no collectives.
"""

import numpy as np

B, T, V, F = 64, 2, 512, 512
NCORES = 8
BPC = B // NCORES  # batches per core
NBLK = V // 128  # 4 row-blocks
N_WARM = 10  # warm-up matmuls (N=512 each) before real work

# packed upper-triangle segment offsets: row-block mi holds cols mi*128..511
SEG_OFF = [0]
for _mi in range(NBLK):
    SEG_OFF.append(SEG_OFF[-1] + V - 128 * _mi)
SEG_TOTAL = SEG_OFF[-1]  # 1280

_NC = None


def _build_nc():
    import concourse.mybir as mybir
    import concourse.tile as tile
    from concourse import bacc

    f32 = mybir.dt.float32
    f16 = mybir.dt.float16

    nc = bacc.Bacc(target_bir_lowering=False)
    z = nc.declare_dram_parameter("z", [BPC, 128, NBLK * V], f16, isOutput=False)
    outp = nc.declare_dram_parameter(
        "outp", [BPC, 128, SEG_TOTAL], f16, isOutput=True
    )

    with tile.TileContext(nc) as tc:
        with (
            tc.tile_pool(name="inp", bufs=3) as inp_pool,
            tc.tile_pool(name="warm", bufs=1) as warm_pool,
            tc.tile_pool(name="psum", bufs=8, space="PSUM") as psum_pool,
            tc.tile_pool(name="outp", bufs=3) as out_pool,
        ):
            # PE warm-up while the first input DMA is in flight
            wt = warm_pool.tile([128, V], f16)
            nc.gpsimd.memset(wt, 0.0)
            wps = psum_pool.tile([128, V], f32, tag="ps")
            for _ in range(N_WARM):
                nc.tensor.matmul(wps, lhsT=wt[:, 0:128], rhs=wt, start=True, stop=True)

            for b in range(BPC):
                zt = inp_pool.tile([128, NBLK * V], f16, tag="z")
                nc.sync.dma_start(out=zt, in_=z[b])
                ot = out_pool.tile([128, SEG_TOTAL], f16, tag="ot")
                for mi in range(NBLK):
                    n_cols = V - 128 * mi
                    ps = psum_pool.tile([128, n_cols], f32, tag="ps")
                    for ki in range(NBLK):
                        base = ki * V + mi * 128
                        nc.tensor.matmul(
                            ps,
                            lhsT=zt[:, base : base + 128],
                            rhs=zt[:, base : (ki + 1) * V],
                            start=(ki == 0),
                            stop=(ki == NBLK - 1),
                        )
                    seg = ot[:, SEG_OFF[mi] : SEG_OFF[mi] + n_cols]
                    if mi % 2 == 0:
                        nc.vector.tensor_copy(out=seg, in_=ps)
                    else:
                        nc.scalar.copy(out=seg, in_=ps)
                nc.scalar.dma_start(out=outp[b], in_=ot)
    if not nc.is_finalized():
        nc.finalize()
    return nc


def _get_nc():
    global _NC
    if _NC is None:
        _NC = _build_nc()
    return _NC


def _prep_shards(x: np.ndarray) -> np.ndarray:
    x = np.ascontiguousarray(np.asarray(x, dtype=np.float32))
    x0 = x[:, 0]  # (B, V, F)
    ss = np.einsum("bvf,bvf->v", x0, x0, optimize=True)
    inv_n = (1.0 / np.sqrt(ss)).astype(np.float32)
    y = x0 * inv_n[None, :, None]
    # z[b, p, ki*512 + v] = y[b, v, ki*128 + p]: each batch is one
    # [128 partitions x 4096B-contiguous] DMA on device
    z = y.reshape(B, V, NBLK, 128).transpose(0, 3, 2, 1).reshape(B, 128, NBLK * V)
    return np.ascontiguousarray(z.astype(np.float16))


def kernel(x: np.ndarray, _trace: bool = False, _trace_out: list | None = None):
    from concourse.bass_utils import run_bass_kernel_spmd

    z = _prep_shards(x)
    nc = _get_nc()
    in_maps = [{"z": z[c * BPC : (c + 1) * BPC]} for c in range(NCORES)]
    res = run_bass_kernel_spmd(
        nc, in_maps, core_ids=list(range(NCORES)), trace=_trace
    )
    if _trace_out is not None:
        _trace_out.append(res)
    packed = np.concatenate(
        [np.asarray(res.results[c]["outp"]) for c in range(NCORES)], axis=0
    )  # (B, 128, 1280) fp16
    full = np.empty((B, V, V), dtype=np.float32)
    for mi in range(NBLK):
        full[:, mi * 128 : (mi + 1) * 128, mi * 128 :] = packed[
            :, :, SEG_OFF[mi] : SEG_OFF[mi + 1]
        ].astype(np.float32)
    # mirror the upper block-triangle down
    for mi in range(NBLK):
        for nj in range(mi + 1, NBLK):
            full[:, nj * 128 : (nj + 1) * 128, mi * 128 : (mi + 1) * 128] = (
                np.swapaxes(
                    full[:, mi * 128 : (mi + 1) * 128, nj * 128 : (nj + 1) * 128],
                    1,
                    2,
                )
            )
    return full


# revision 4
# speedup vs baseline: 1.3357x; 1.1062x over previous
"""Batched normalized-gram kernel for 8 TRN2 NeuronCores.

reference:  x (64, 2, 512, 512) fp32
    x0 = x[:, 0]                               (B=64, V=512, F=512)
    n  = sqrt(sum(x0^2, axis=(0, 2)))          (V,)
    out[b] = (x0[b] @ x0[b].T) / outer(n, n)   (B, V, V)

gram[b,i,j]/(n_i n_j) == (x0[b,i,:]/n_i) . (x0[b,j,:]/n_j), so the host
prescales rows by 1/n once and the device work is a pure batched symmetric
matmul out[b] = y[b] @ y[b].T.

Device-side structure (per core, 8 batches):
  * operands shipped as fp16 — halves input DMA, full-rate PE, fp32 PSUM.
  * output is symmetric: device computes only the upper block-triangle
    (row-block mi covers columns mi*128..511), host mirrors the rest.
  * ONE input DMA per batch (batch 0: two half DMAs so compute starts as
    soon as the first half lands): host pre-interleaves y[b].T into a
    [128, 2048] layout (z[b, p, ki*512+v] = yT[b, ki*128+p, v]) so each
    batch streams as 128 x 4KiB contiguous descriptors.  Keeps the SP
    sequencer (~600ns config per DMA) far ahead of the PE.
  * input prefetch 5 batches deep — absorbs the ~3us DMA delivery latency
    (config+DGE+transfer+sem) without ever stalling the PE.
  * packed-triangle output staged in SBUF as fp16 (halves output DMA),
    one DMA per batch (last batch: two, so the final transfer is small),
    unpacked/mirrored on host.
  * PSUM->SBUF cast-copies split DVE (mi=0,2) / ACT (mi=1,3).
  * dummy warm-up matmuls on a zeroed tile run while batch 0's input DMA
    is in flight: PE is busy from the first user instruction, HAM
    un-throttles to 2.4 GHz by the time real matmuls begin.

Sharding: data-parallel over batch — 8 batches per core, no collectives.
"""

import numpy as np

B, T, V, F = 64, 2, 512, 512
NCORES = 8
BPC = B // NCORES  # batches per core
NBLK = V // 128  # 4 row-blocks
N_WARM = 8  # warm-up matmuls (N=512 each) before real work

# packed upper-triangle segment offsets: row-block mi holds cols mi*128..511
SEG_OFF = [0]
for _mi in range(NBLK):
    SEG_OFF.append(SEG_OFF[-1] + V - 128 * _mi)
SEG_TOTAL = SEG_OFF[-1]  # 1280

_NC = None


def _build_nc():
    import concourse.mybir as mybir
    import concourse.tile as tile
    from concourse import bacc

    f32 = mybir.dt.float32
    f16 = mybir.dt.float16

    nc = bacc.Bacc(target_bir_lowering=False)
    z = nc.declare_dram_parameter("z", [BPC, 128, NBLK * V], f16, isOutput=False)
    outp = nc.declare_dram_parameter(
        "outp", [BPC, 128, SEG_TOTAL], f16, isOutput=True
    )

    def copy_seg(ot, mi, ps):
        seg = ot[:, SEG_OFF[mi] : SEG_OFF[mi] + (V - 128 * mi)]
        if mi % 2 == 0:
            nc.vector.tensor_copy(out=seg, in_=ps)
        else:
            nc.scalar.copy(out=seg, in_=ps)

    with tile.TileContext(nc) as tc:
        with (
            tc.tile_pool(name="boot", bufs=1) as boot_pool,
            tc.tile_pool(name="inp", bufs=5) as inp_pool,
            tc.tile_pool(name="psum", bufs=8, space="PSUM") as psum_pool,
            tc.tile_pool(name="outp", bufs=3) as out_pool,
        ):
            # PE warm-up while batch 0's input DMA is in flight
            wt = boot_pool.tile([128, V], f16, tag="warm")
            nc.gpsimd.memset(wt, 0.0)
            wps = psum_pool.tile([128, V], f32, tag="ps")
            for _ in range(N_WARM):
                nc.tensor.matmul(wps, lhsT=wt[:, 0:128], rhs=wt, start=True, stop=True)

            # batch 0: two half-tiles, ki-outer matmuls — first 8 matmuls
            # only need the first half of the input
            zh = []
            for h in range(2):
                t = boot_pool.tile([128, 2 * V], f16, tag=f"z0{h}")
                nc.sync.dma_start(out=t, in_=z[0, :, h * 2 * V : (h + 1) * 2 * V])
                zh.append(t)
            ps0 = [
                psum_pool.tile([128, V - 128 * mi], f32, tag="ps", name=f"ps0_{mi}")
                for mi in range(NBLK)
            ]
            ot0 = out_pool.tile([128, SEG_TOTAL], f16, tag="ot")
            for ki in range(NBLK):
                src = zh[ki // 2]
                for mi in range(NBLK):
                    base = (ki % 2) * V + mi * 128
                    nc.tensor.matmul(
                        ps0[mi],
                        lhsT=src[:, base : base + 128],
                        rhs=src[:, base : (ki % 2) * V + V],
                        start=(ki == 0),
                        stop=(ki == NBLK - 1),
                    )
            for mi in range(NBLK):
                copy_seg(ot0, mi, ps0[mi])
            nc.scalar.dma_start(out=outp[0], in_=ot0)

            for b in range(1, BPC):
                last = b == BPC - 1
                zt = inp_pool.tile([128, NBLK * V], f16, tag="z")
                nc.sync.dma_start(out=zt, in_=z[b])
                ot = out_pool.tile([128, SEG_TOTAL], f16, tag="ot")
                for mi in range(NBLK):
                    n_cols = V - 128 * mi
                    ps = psum_pool.tile([128, n_cols], f32, tag="ps")
                    for ki in range(NBLK):
                        base = ki * V + mi * 128
                        nc.tensor.matmul(
                            ps,
                            lhsT=zt[:, base : base + 128],
                            rhs=zt[:, base : (ki + 1) * V],
                            start=(ki == 0),
                            stop=(ki == NBLK - 1),
                        )
                    copy_seg(ot, mi, ps)
                    if last and mi == 1:
                        # ship segs 0-1 early so the post-loop tail DMA is small
                        nc.scalar.dma_start(
                            out=outp[b, :, : SEG_OFF[2]], in_=ot[:, : SEG_OFF[2]]
                        )
                if last:
                    nc.scalar.dma_start(
                        out=outp[b, :, SEG_OFF[2] :], in_=ot[:, SEG_OFF[2] :]
                    )
                else:
                    nc.scalar.dma_start(out=outp[b], in_=ot)
    if not nc.is_finalized():
        nc.finalize()
    return nc


def _get_nc():
    global _NC
    if _NC is None:
        _NC = _build_nc()
    return _NC


def _prep_shards(x: np.ndarray) -> np.ndarray:
    x = np.ascontiguousarray(np.asarray(x, dtype=np.float32))
    x0 = x[:, 0]  # (B, V, F)
    ss = np.einsum("bvf,bvf->v", x0, x0, optimize=True)
    inv_n = (1.0 / np.sqrt(ss)).astype(np.float32)
    y = x0 * inv_n[None, :, None]
    # z[b, p, ki*512 + v] = y[b, v, ki*128 + p]: each batch is one
    # [128 partitions x 4096B-contiguous] DMA on device
    z = y.reshape(B, V, NBLK, 128).transpose(0, 3, 2, 1).reshape(B, 128, NBLK * V)
    return np.ascontiguousarray(z.astype(np.float16))


def kernel(x: np.ndarray, _trace: bool = False, _trace_out: list | None = None):
    from concourse.bass_utils import run_bass_kernel_spmd

    z = _prep_shards(x)
    nc = _get_nc()
    in_maps = [{"z": z[c * BPC : (c + 1) * BPC]} for c in range(NCORES)]
    res = run_bass_kernel_spmd(
        nc, in_maps, core_ids=list(range(NCORES)), trace=_trace
    )
    if _trace_out is not None:
        _trace_out.append(res)
    packed = np.concatenate(
        [np.asarray(res.results[c]["outp"]) for c in range(NCORES)], axis=0
    )  # (B, 128, 1280) fp16
    full = np.empty((B, V, V), dtype=np.float32)
    for mi in range(NBLK):
        full[:, mi * 128 : (mi + 1) * 128, mi * 128 :] = packed[
            :, :, SEG_OFF[mi] : SEG_OFF[mi + 1]
        ].astype(np.float32)
    # mirror the upper block-triangle down
    for mi in range(NBLK):
        for nj in range(mi + 1, NBLK):
            full[:, nj * 128 : (nj + 1) * 128, mi * 128 : (mi + 1) * 128] = (
                np.swapaxes(
                    full[:, mi * 128 : (mi + 1) * 128, nj * 128 : (nj + 1) * 128],
                    1,
                    2,
                )
            )
    return full


# revision 7
# speedup vs baseline: 1.3371x; 1.0010x over previous
"""Batched normalized-gram kernel for 8 TRN2 NeuronCores.

reference:  x (64, 2, 512, 512) fp32
    x0 = x[:, 0]                               (B=64, V=512, F=512)
    n  = sqrt(sum(x0^2, axis=(0, 2)))          (V,)
    out[b] = (x0[b] @ x0[b].T) / outer(n, n)   (B, V, V)

gram[b,i,j]/(n_i n_j) == (x0[b,i,:]/n_i) . (x0[b,j,:]/n_j), so the host
prescales rows by 1/n once and the device work is a pure batched symmetric
matmul out[b] = y[b] @ y[b].T.

Device-side structure (per core, 8 batches):
  * operands shipped as fp16 — halves input DMA, full-rate PE, fp32 PSUM.
  * output is symmetric: device computes only the upper block-triangle
    (row-block mi covers columns mi*128..511), host mirrors the rest.
  * ONE input DMA per batch (batch 0: two half DMAs so compute starts as
    soon as the first half lands): host pre-interleaves y[b].T into a
    [128, 2048] layout (z[b, p, ki*512+v] = yT[b, ki*128+p, v]) so each
    batch streams as 128 x 4KiB contiguous descriptors.  Keeps the SP
    sequencer (~600ns config per DMA) far ahead of the PE.
  * input prefetch 5 batches deep — absorbs the ~3us DMA delivery latency
    (config+DGE+transfer+sem) without ever stalling the PE.
  * packed-triangle output staged in SBUF as fp16 (halves output DMA),
    one DMA per batch (last batch: two, so the final transfer is small),
    unpacked/mirrored on host.
  * PSUM->SBUF cast-copies split DVE (mi=0,2) / ACT (mi=1,3).
  * dummy warm-up matmuls on a zeroed tile run while batch 0's input DMA
    is in flight: PE is busy from the first user instruction, HAM
    un-throttles to 2.4 GHz by the time real matmuls begin.

Sharding: data-parallel over batch — 8 batches per core, no collectives.
"""

import numpy as np

B, T, V, F = 64, 2, 512, 512
NCORES = 8
BPC = B // NCORES  # batches per core
NBLK = V // 128  # 4 row-blocks
N_WARM = 6  # warm-up matmuls (N=512 each) before real work

# packed upper-triangle segment offsets: row-block mi holds cols mi*128..511
SEG_OFF = [0]
for _mi in range(NBLK):
    SEG_OFF.append(SEG_OFF[-1] + V - 128 * _mi)
SEG_TOTAL = SEG_OFF[-1]  # 1280

_NC = None


def _build_nc():
    import concourse.mybir as mybir
    import concourse.tile as tile
    from concourse import bacc

    f32 = mybir.dt.float32
    f16 = mybir.dt.float16

    nc = bacc.Bacc(target_bir_lowering=False)
    z = nc.declare_dram_parameter("z", [BPC, 128, NBLK * V], f16, isOutput=False)
    outp = nc.declare_dram_parameter(
        "outp", [BPC, 128, SEG_TOTAL], f16, isOutput=True
    )

    def copy_seg(ot, mi, ps):
        seg = ot[:, SEG_OFF[mi] : SEG_OFF[mi] + (V - 128 * mi)]
        if mi % 2 == 0:
            nc.vector.tensor_copy(out=seg, in_=ps)
        else:
            nc.scalar.copy(out=seg, in_=ps)

    with tile.TileContext(nc) as tc:
        with (
            tc.tile_pool(name="boot", bufs=1) as boot_pool,
            tc.tile_pool(name="inp", bufs=5) as inp_pool,
            tc.tile_pool(name="psum", bufs=8, space="PSUM") as psum_pool,
            tc.tile_pool(name="outp", bufs=3) as out_pool,
        ):
            # PE warm-up while batch 0's input DMA is in flight
            wt = boot_pool.tile([128, V], f16, tag="warm")
            nc.gpsimd.memset(wt, 0.0)
            wps = psum_pool.tile([128, V], f32, tag="ps")
            for _ in range(N_WARM):
                nc.tensor.matmul(wps, lhsT=wt[:, 0:128], rhs=wt, start=True, stop=True)

            # batch 0: four quarter-tiles, ki-outer matmuls — the first 4
            # matmuls only need the first quarter of the input
            zh = []
            for h in range(NBLK):
                t = boot_pool.tile([128, V], f16, tag=f"z0{h}", name=f"z0{h}")
                nc.sync.dma_start(out=t, in_=z[0, :, h * V : (h + 1) * V])
                zh.append(t)
            ps0 = [
                psum_pool.tile([128, V - 128 * mi], f32, tag="ps", name=f"ps0_{mi}")
                for mi in range(NBLK)
            ]
            ot0 = out_pool.tile([128, SEG_TOTAL], f16, tag="ot")
            for ki in range(NBLK):
                src = zh[ki]
                for mi in range(NBLK):
                    nc.tensor.matmul(
                        ps0[mi],
                        lhsT=src[:, mi * 128 : (mi + 1) * 128],
                        rhs=src[:, mi * 128 :],
                        start=(ki == 0),
                        stop=(ki == NBLK - 1),
                    )
            for mi in range(NBLK):
                copy_seg(ot0, mi, ps0[mi])
            nc.scalar.dma_start(out=outp[0], in_=ot0)

            for b in range(1, BPC):
                last = b == BPC - 1
                zt = inp_pool.tile([128, NBLK * V], f16, tag="z")
                nc.sync.dma_start(out=zt, in_=z[b])
                ot = out_pool.tile([128, SEG_TOTAL], f16, tag="ot")
                for mi in range(NBLK):
                    n_cols = V - 128 * mi
                    ps = psum_pool.tile([128, n_cols], f32, tag="ps")
                    for ki in range(NBLK):
                        base = ki * V + mi * 128
                        nc.tensor.matmul(
                            ps,
                            lhsT=zt[:, base : base + 128],
                            rhs=zt[:, base : (ki + 1) * V],
                            start=(ki == 0),
                            stop=(ki == NBLK - 1),
                        )
                    if last:
                        # last batch: every copy on DVE, one output DMA per
                        # segment so the post-loop tail is a single tiny
                        # transfer on the otherwise-idle Sync ring
                        seg = ot[:, SEG_OFF[mi] : SEG_OFF[mi] + n_cols]
                        nc.vector.tensor_copy(out=seg, in_=ps)
                        if mi < NBLK - 1:
                            nc.scalar.dma_start(
                                out=outp[b, :, SEG_OFF[mi] : SEG_OFF[mi + 1]],
                                in_=seg,
                            )
                        else:
                            nc.sync.dma_start(
                                out=outp[b, :, SEG_OFF[mi] : SEG_OFF[mi + 1]],
                                in_=seg,
                            )
                    else:
                        copy_seg(ot, mi, ps)
                if not last:
                    nc.scalar.dma_start(out=outp[b], in_=ot)
    if not nc.is_finalized():
        nc.finalize()
    return nc


def _get_nc():
    global _NC
    if _NC is None:
        _NC = _build_nc()
    return _NC


def _prep_shards(x: np.ndarray) -> np.ndarray:
    x = np.ascontiguousarray(np.asarray(x, dtype=np.float32))
    x0 = x[:, 0]  # (B, V, F)
    ss = np.einsum("bvf,bvf->v", x0, x0, optimize=True)
    inv_n = (1.0 / np.sqrt(ss)).astype(np.float32)
    y = x0 * inv_n[None, :, None]
    # z[b, p, ki*512 + v] = y[b, v, ki*128 + p]: each batch is one
    # [128 partitions x 4096B-contiguous] DMA on device
    z = y.reshape(B, V, NBLK, 128).transpose(0, 3, 2, 1).reshape(B, 128, NBLK * V)
    return np.ascontiguousarray(z.astype(np.float16))


def kernel(x: np.ndarray, _trace: bool = False, _trace_out: list | None = None):
    from concourse.bass_utils import run_bass_kernel_spmd

    z = _prep_shards(x)
    nc = _get_nc()
    in_maps = [{"z": z[c * BPC : (c + 1) * BPC]} for c in range(NCORES)]
    res = run_bass_kernel_spmd(
        nc, in_maps, core_ids=list(range(NCORES)), trace=_trace
    )
    if _trace_out is not None:
        _trace_out.append(res)
    packed = np.concatenate(
        [np.asarray(res.results[c]["outp"]) for c in range(NCORES)], axis=0
    )  # (B, 128, 1280) fp16
    full = np.empty((B, V, V), dtype=np.float32)
    for mi in range(NBLK):
        full[:, mi * 128 : (mi + 1) * 128, mi * 128 :] = packed[
            :, :, SEG_OFF[mi] : SEG_OFF[mi + 1]
        ].astype(np.float32)
    # mirror the upper block-triangle down
    for mi in range(NBLK):
        for nj in range(mi + 1, NBLK):
            full[:, nj * 128 : (nj + 1) * 128, mi * 128 : (mi + 1) * 128] = (
                np.swapaxes(
                    full[:, mi * 128 : (mi + 1) * 128, nj * 128 : (nj + 1) * 128],
                    1,
                    2,
                )
            )
    return full
